# revision 50
# baseline (speedup 1.0000x reference)
"""Distributed Trainium2 kernel for a 16-head self-attention block.

Model (reference):
    qkv = x @ W_qkv + b_qkv ; q,k,v per 16 heads (head_dim 64)
    attn = softmax(q k^T / 8) ; out = (attn @ v heads concat) @ W_out + b_out
Shapes: x [2, 2048, 1024], W_qkv [1024, 3072], W_out [1024, 1024].

Sharding (8 NeuronCores): 2 batch groups x 4 cores; each core owns one batch
element and 4 of the 16 heads (Megatron-style column-parallel QKV + row-
parallel out-proj). Each core computes its partial out-projection
[2048, 1024] in bf16; the host sums the 4 partials per batch and adds the
output bias (the V bias folds out of attention exactly, so bv@W_out is
added to the host-side bias).

Performance notes (perfetto-trace driven; 344.9us v1 -> 190.3us):
  * The ScalarE exp over all 4 heads' scores (16.8M elems/core) is the hard
    floor (~1.1us per [128,1024] tile, 128 tiles). Everything else must
    hide under that stream: the kernel is one software-pipelined sequence
    of per-(head, q-half) "streams", where stream N runs scores+exp for
    head N on PE/ACT while the PV matmuls of head N-1 (consuming its
    pinned exp tiles) fill the PE, with projection / out-projection blocks
    spliced in as PE filler.  Each stream is emitted in two priority bands
    (scores+exp first, PV/fillers second) and the next stream's first 4
    scores are hoisted into the current stream's band 1, so the per-engine
    ready heaps keep the exp stream running across stream boundaries.
  * Scores matmuls use the full-height [128,128] K chunk as stationary
    against a zero-padded q tile (the other head's partitions are zero).
    Sub-128-row stationaries don't register as PE activity in the HAM
    clock gate, which otherwise throttles the whole kernel to 4/8 clock
    (1.2 GHz); full-height stationaries also get FWL weight loads.
  * Q/K projections run as fp8-e4m3 DoubleRow matmuls (256-row reduction
    chunks; weights host-prescaled by 64 into fp8's normal range, scale
    folded into the exp's free affine).  V is computed from the same fp8 x
    chunks against bf16 Wv, so no bf16 copy of x is shipped at all (3.5 MB
    total input).  Measured output rel err 1.25e-2 vs the 2e-2 gate.
  * exp tiles are [128, 1024] (FD=1024): halves per-instruction ACT
    overhead (~350 cyc) vs FD=512.
  * PSUM (8 banks, tiles bank-padded): scores 2x[128,1024] (4 banks) + PV
    accumulators 2x[128,4,65] (2 banks) + one shared 2-slot aux tag for
    proj / transpose / out-proj outputs (2 banks).  PV accumulation groups
    sharing a bank run strictly one-after-another (q4-major order) because
    a matmul with start=True clears the has_written bits of its whole bank.
  * softmax runs without max-subtraction: scores/8 are bounded ~|2.5|.
  * V carries a ones column so the PV matmul emits the softmax denominator
    as its column 64 (per query partition); normalization is a per-
    partition reciprocal+scale on the PV output before the transpose.
  * The tail (last head's PV + finish + last 8 out-proj blocks) borrows
    the then-idle sc-pool PSUM slots and alternates evictions between
    ScalarE and VectorE.

Device dataflow per core (layouts chosen so no transposes are needed
except the [q,d]->[d,q] flip of the per-head attention output):
  x^T fp8-packed [128, 2, n] chunks ->
  Q^T,K^T [256, 2048] = 64*(Wq^T x^T + b) bf16 (DoubleRow, bias on
    eviction);  V [2048, 4, 65] bf16 from fp8 x against bf16 Wv
  per (head, q-half 1024): per kt (128 k rows): scores^T [128, 1024] PSUM,
    exp(psum/(8*64^2)) -> E [128, 1024] bf16 (pinned for the head);
  next stream: per q-chunk of 128: U[q 128, 65] += E(kt)^T V(kt) over kt;
  normalize per q-chunk with reciprocal of column 64, PE-transpose to
  U^T[64, 128] (batched 4 to an aux bank), paste into ut;
  out partial rows [128, 1024] = (ut chunk)^T @ Wo, evicted bf16, DMA out.
"""

import contextlib

import numpy as np

import concourse.bacc as bacc
import concourse.mybir as mybir
import concourse.tile as tile
from concourse import bass_utils

F32 = mybir.dt.float32
BF16 = mybir.dt.bfloat16
FP8 = mybir.dt.float8e4
AF = mybir.ActivationFunctionType

WSCALE = 64.0     # Wq/Wk host pre-scale into fp8 e4m3's normal range

S = 2048          # sequence length (one batch element per core)
E = 1024          # embedding dim
HD = 64           # head dim
NH = 4            # heads per core
DQ = NH * HD      # per-core q/k/v width (256)
ET = E // 128     # embedding tiles (8)
ST = S // 128     # sequence tiles (16)

_CACHED = {}


def build_core_graph():
    MD = BF16

    nc = bacc.Bacc("TRN2", target_bir_lowering=False, debug=False, num_devices=8)

    # fp8 DoubleRow operands: 256-row chunk c packed as [128, 2, n] with
    # [p, j] holding logical row 256c + 128j + p; x^T split by q-half so
    # the first-needed MB arrives first.  The V projection reuses the fp8
    # x chunks as plain [128,128] stationaries (xdr[h][c][:, j] holds
    # x^T rows 256c+128j..+127), so no bf16 copy of x is shipped at all.
    x8a_d = nc.dram_tensor("x8a", [512, S], FP8, kind="ExternalInput")
    x8b_d = nc.dram_tensor("x8b", [512, S], FP8, kind="ExternalInput")
    wq8_d = nc.dram_tensor("wq8", [512, 2 * DQ], FP8, kind="ExternalInput")
    wk8_d = nc.dram_tensor("wk8", [512, 2 * DQ], FP8, kind="ExternalInput")
    wv_d = nc.dram_tensor("wv", [E, DQ], MD, kind="ExternalInput")
    bq_d = nc.dram_tensor("bq", [DQ, 1], F32, kind="ExternalInput")
    bk_d = nc.dram_tensor("bk", [DQ, 1], F32, kind="ExternalInput")
    wo_d = nc.dram_tensor("wo", [DQ, E], MD, kind="ExternalInput")
    out_d = nc.dram_tensor("out", [S, E], MD, kind="ExternalOutput")

    with tile.TileContext(nc) as tc:
        with contextlib.ExitStack() as ctx:
            # ---- persistent SBUF ------------------------------------------
            pers = ctx.enter_context(tc.tile_pool(name="pers", bufs=1))

            def ptile(shape, dtype, nm):
                return pers.tile(shape, dtype, tag=nm, name=nm)

            qt = [ptile([128, S], MD, f"qt{t}") for t in range(2)]
            kt_sb = [ptile([128, S], MD, f"kt{t}") for t in range(2)]
            ut = [ptile([128, S], MD, f"ut{t}") for t in range(2)]
            v_sb = [ptile([128, NH, HD + 1], MD, f"v{st}") for st in range(ST)]
            wo_sb = [ptile([128, E], MD, f"wo{t}") for t in range(2)]
            bq_sb = [ptile([128, 1], F32, f"bq{t}") for t in range(2)]
            bk_sb = [ptile([128, 1], F32, f"bk{t}") for t in range(2)]
            xdr = [[ptile([128, 2, S // 2], FP8, f"x8{j}{c}") for c in range(4)]
                   for j in range(2)]
            wq8 = [ptile([128, 2, DQ], FP8, f"wq8{c}") for c in range(4)]
            wk8 = [ptile([128, 2, DQ], FP8, f"wk8{c}") for c in range(4)]
            wvs = [ptile([128, DQ], MD, f"wv{et}") for et in range(ET)]
            # zero-padded per-parity q tiles: scores use the full-height
            # [128,128] K chunk as stationary (FWL-eligible) against a
            # moving q whose other-head partitions are zero.
            zq = [ptile([128, 1024], MD, f"zq{par}") for par in range(2)]
            ident = ptile([128, 128], MD, "ident")

            from concourse.masks import make_identity
            make_identity(nc, ident[:])
            # DMA completion (sems) is FIFO per issuing ring, so emit in
            # first-need order: fp8 Q/K operands (small) first, then bf16
            # x^T (for V) in 2KB-row halves, then Wv / Wo.
            for c in range(4):
                sl = slice(c * 128, (c + 1) * 128)
                # first q-half quarter-split so the prologue K block can
                # start on chunk c as soon as its first 512 columns land
                nc.sync.dma_start(
                    xdr[0][c][:, :, 0:512],
                    x8a_d[sl, :].rearrange("p (j n) -> p j n", j=2)[:, :, 0:512])
                nc.sync.dma_start(
                    wk8[c][:], wk8_d[sl, :].rearrange("p (j m) -> p j m", j=2))
                nc.sync.dma_start(
                    wq8[c][:], wq8_d[sl, :].rearrange("p (j m) -> p j m", j=2))
            for c in range(4):
                sl = slice(c * 128, (c + 1) * 128)
                nc.sync.dma_start(
                    xdr[0][c][:, :, 512:1024],
                    x8a_d[sl, :].rearrange("p (j n) -> p j n", j=2)[:, :, 512:1024])
            for t in range(2):
                nc.sync.dma_start(bq_sb[t][:], bq_d[t * 128:(t + 1) * 128, :])
                nc.sync.dma_start(bk_sb[t][:], bk_d[t * 128:(t + 1) * 128, :])
            nc.vector.memset(zq[0][:], 0.0)
            nc.vector.memset(zq[1][:], 0.0)
            for st in range(ST):
                nc.vector.memset(v_sb[st][:, :, HD:HD + 1], 1.0)

            # ---- cycling pools --------------------------------------------
            # PSUM: sc 2x2 banks + pv 2x1 + aux 2x1 = 8 banks exactly.
            sc_ps = ctx.enter_context(
                tc.tile_pool(name="sc_ps", bufs=2, space="PSUM"))
            pv_ps = ctx.enter_context(
                tc.tile_pool(name="pv_ps", bufs=1, space="PSUM"))
            aux_ps = ctx.enter_context(
                tc.tile_pool(name="aux_ps", bufs=2, space="PSUM"))
            # e tiles: 16 of head N-1 pinned by PV (batch-released only after
            # PV's last matmul) + 16 of head N in flight + ~8 of head N+1 so
            # the exp stream doesn't stall at stream boundaries.
            e_pool = ctx.enter_context(tc.tile_pool(name="e_sb", bufs=48))
            un_pool = ctx.enter_context(tc.tile_pool(name="un_sb", bufs=8))
            rc_pool = ctx.enter_context(tc.tile_pool(name="rc_sb", bufs=8))
            o_pool = ctx.enter_context(tc.tile_pool(name="o_sb", bufs=4))

            def aux_tile(shape, dtype=F32):
                return aux_ps.tile(shape, dtype, tag="aux", name="aux")

            # ---- building blocks ------------------------------------------
            def qk_block(t, qb, w_tiles, b_sb, dst):
                """dst[t][:, qb*512:+512] = WSCALE*(w^T x^T + b) for one
                128-row chunk t of Q^T or K^T (fp8 DoubleRow matmuls)."""
                tsl = slice(t * 128, (t + 1) * 128)
                qsl = slice(qb * 512, (qb + 1) * 512)
                hsl = slice((qb % 2) * 512, (qb % 2) * 512 + 512)
                ps = aux_tile([128, 512])
                for c in range(4):
                    nc.tensor.matmul(ps[:], w_tiles[c][:, :, tsl],
                                     xdr[qb // 2][c][:, :, hsl],
                                     start=(c == 0), stop=(c == 3),
                                     perf_mode=mybir.MatmulPerfMode.DoubleRow)
                nc.vector.tensor_scalar_add(dst[t][:, qsl], ps[:], b_sb[t][:])

            def v_block(st):
                """v_sb[st][:, :, 0:64] = (x^T)^T wv for one 128-row chunk
                (fp8 x stationary, bf16 wv moving)."""
                hsl = slice((st % 8) * 128, (st % 8) * 128 + 128)
                ps = aux_tile([128, DQ])
                for c in range(4):
                    for j in range(2):
                        nc.tensor.matmul(ps[:], xdr[st // 8][c][:, j, hsl],
                                         wvs[2 * c + j][:],
                                         start=(c == 0 and j == 0),
                                         stop=(c == 3 and j == 1))
                nc.vector.tensor_copy(
                    v_sb[st][:, :, 0:HD],
                    ps[:].rearrange("p (h d) -> p h d", h=NH))

            def load_zq(h, qh):
                """Stage head h's q-half into its parity's zero-padded tile."""
                t, po = h // 2, (h % 2) * HD
                nc.vector.tensor_copy(
                    zq[h % 2][po:po + HD, :],
                    qt[t][po:po + HD, qh * 1024:(qh + 1) * 1024])

            def scores_exp(h, qh, kt):
                """scores^T [128 k, 1024 q] for head h, then exp -> bf16.
                Stationary is the full-height K chunk (FWL-eligible); the
                other head's rows multiply zq's zero partitions."""
                t = h // 2
                ksl = slice(kt * 128, (kt + 1) * 128)
                sc = sc_ps.tile([128, 1024], F32, tag="sc", name="sc")
                for half in range(2):
                    nc.tensor.matmul(sc[:, half * 512:(half + 1) * 512],
                                     kt_sb[t][:, ksl],
                                     zq[h % 2][:, half * 512:(half + 1) * 512],
                                     start=True, stop=True)
                e_sb = e_pool.tile([128, 1024], MD, tag="e", name="e")
                nc.scalar.activation(e_sb[:], sc[:], AF.Exp,
                                     scale=0.125 / (WSCALE * WSCALE))
                return e_sb

            def pv_8(h, e_tiles, pvp, it):
                """8 PV matmuls, q4-major: global index i = 8*it + j maps to
                accumulation group q4 = i//16 (strictly sequential per PSUM
                bank) over kt = i%16."""
                for j in range(8):
                    i = 8 * it + j
                    q4, kt = i // 16, i % 16
                    nc.tensor.matmul(pvp[q4 // 4][:, q4 % 4, :],
                                     e_tiles[kt][:, q4 * 128:(q4 + 1) * 128],
                                     v_sb[kt][:, h, :],
                                     start=(kt == 0), stop=(kt == ST - 1))

            def finish_group(h, qh, pvp, grp):
                """Normalize 4 q-chunks of U, transpose into one aux bank,
                paste into ut."""
                t, po = h // 2, (h % 2) * HD
                tp = aux_tile([HD, 512], MD)
                for q4 in range(4):
                    rc = rc_pool.tile([128, 1], F32, tag="rc", name="rc")
                    nc.vector.reciprocal(rc[:], pvp[grp][:, q4, HD:HD + 1])
                    un = un_pool.tile([128, HD], MD, tag="un", name="un")
                    nc.vector.tensor_scalar_mul(un[:], pvp[grp][:, q4, 0:HD],
                                                rc[:])
                    nc.tensor.transpose(tp[:, q4 * 128:(q4 + 1) * 128],
                                        un[:], ident[:, 0:128])
                ssl = slice(qh * 1024 + grp * 512, qh * 1024 + (grp + 1) * 512)
                nc.vector.tensor_copy(ut[t][po:po + HD, ssl], tp[:])

            def out_block(st):
                """out rows [st*128, +128) = (ut chunk)^T @ Wo, bf16."""
                ssl = slice(st * 128, (st + 1) * 128)
                o_sb = o_pool.tile([128, E], MD, tag="o", name="o")
                for ob in range(2):
                    osl = slice(ob * 512, (ob + 1) * 512)
                    op = aux_tile([128, 512])
                    for t in range(2):
                        nc.tensor.matmul(op[:], ut[t][:, ssl], wo_sb[t][:, osl],
                                         start=(t == 0), stop=(t == 1))
                    nc.vector.tensor_copy(o_sb[:, osl], op[:])
                nc.sync.dma_start(out_d[ssl, :], o_sb[:])

            def out_block_tail(st, eng):
                """Tail variant: the exp stream is over, so borrow a wide
                sc-pool PSUM slot and evict on whichever of ScalarE/VectorE
                is named (both are otherwise idle in the tail)."""
                ssl = slice(st * 128, (st + 1) * 128)
                o_sb = o_pool.tile([128, E], MD, tag="o", name="o")
                op = sc_ps.tile([128, 1024], F32, tag="sc", name="sc")
                for ob in range(2):
                    osl = slice(ob * 512, (ob + 1) * 512)
                    for t in range(2):
                        nc.tensor.matmul(op[:, osl], ut[t][:, ssl],
                                         wo_sb[t][:, osl],
                                         start=(t == 0), stop=(t == 1))
                if eng == "scalar":
                    nc.scalar.copy(o_sb[:], op[:])
                else:
                    nc.vector.tensor_copy(o_sb[:], op[:])
                nc.sync.dma_start(out_d[ssl, :], o_sb[:])

            # ---- emission: software-pipelined streams ---------------------
            # Stream (h, qh) = 16 iterations of [scores+exp(h), 8x PV(prev
            # head), filler?], then finish(prev head).  Order of streams:
            # qh0 h0..3, qh1 h0..3, then a tail that drains the last head's
            # PV interleaved with the last out-proj blocks.  Fillers are
            # placed so every projection block is emitted (and thus runs)
            # at least one stream before its first consumer, and AFTER the
            # scores/PV of their iteration so the exp stream has priority.
            qk_block(0, 0, wk8, bk_sb, kt_sb)      # K k 0..511 of t=0
            qk_block(0, 0, wq8, bq_sb, qt)         # Q q 0..511 of t=0
            qk_block(0, 1, wq8, bq_sb, qt)         # Q q 512..1023

            for c in range(4):
                sl = slice(c * 128, (c + 1) * 128)
                nc.sync.dma_start(
                    xdr[1][c][:], x8b_d[sl, :].rearrange("p (j n) -> p j n", j=2))
            for et in range(ET):
                sl = slice(et * 128, (et + 1) * 128)
                nc.sync.dma_start(wvs[et][:], wv_d[sl, :])
            for t in range(2):
                nc.sync.dma_start(wo_sb[t][:], wo_d[t * 128:(t + 1) * 128, :])

            # Streams are emitted in two priority bands: band 1 carries the
            # scores+exp clock (plus the QK blocks those scores need,
            # spliced in just before their first consumer), band 2 carries
            # the previous head's PV, V blocks and out-proj.  To keep the
            # exp stream running across stream boundaries (where band-2
            # leftovers otherwise outrank the next stream's scores), each
            # stream's first 4 scores+exp are hoisted into the previous
            # stream's band 1 ("head start").
            HEAD = 4
            started = {}

            def start_stream(h, qh):
                load_zq(h, qh)
                started[(h, qh)] = [scores_exp(h, qh, it) for it in range(HEAD)]

            def stream(h, qh, prev, fillers, band1_fillers=None, nxt=None):
                """prev = (ph, pqh, p_etiles, p_pvp) or None."""
                band1_fillers = dict(band1_fillers or {})
                e_tiles = started.pop((h, qh))
                if prev is not None:
                    ph, pqh, p_etiles, p_pvp = prev
                for it in range(HEAD, ST):
                    if it in band1_fillers:
                        band1_fillers.pop(it)()
                    e_tiles.append(scores_exp(h, qh, it))
                if nxt is not None:
                    start_stream(*nxt)
                fi = 0
                for it in range(ST):
                    if prev is not None:
                        pv_8(ph, p_etiles, p_pvp, it)
                    if fi < len(fillers):
                        fillers[fi]()
                        fi += 1
                while fi < len(fillers):
                    fillers[fi]()
                    fi += 1
                if prev is not None:
                    finish_group(ph, pqh, p_pvp, 0)
                    finish_group(ph, pqh, p_pvp, 1)
                pvp = [pv_ps.tile([128, 4, HD + 1], F32, tag=f"pv{g}",
                                  name=f"pv{g}") for g in range(2)]
                return (h, qh, e_tiles, pvp)

            start_stream(0, 0)
            # Band-1 filler keys: each QK block lands just before its first
            # band-1 consumer (scores of this stream or the next stream's
            # hoisted head).
            f_s00_b1 = {
                4: lambda: qk_block(0, 1, wk8, bk_sb, kt_sb),
                8: lambda: qk_block(0, 2, wk8, bk_sb, kt_sb),
                12: lambda: qk_block(0, 3, wk8, bk_sb, kt_sb),
            }
            # K t=1 qb2/qb3 feed only stream (2,0)'s body scores kt>=8, so
            # they move to that stream's own band 1 -- this evens the PE
            # load between streams (1,0) and (2,0).
            f_s10_b1 = {
                4: lambda: qk_block(1, 0, wk8, bk_sb, kt_sb),
                7: lambda: qk_block(1, 1, wk8, bk_sb, kt_sb),
                10: lambda: qk_block(1, 0, wq8, bq_sb, qt),
                13: lambda: qk_block(1, 1, wq8, bq_sb, qt),
            }
            f_s20_b1 = {
                4: lambda: qk_block(1, 2, wk8, bk_sb, kt_sb),
                7: lambda: qk_block(0, 2, wq8, bq_sb, qt),
                10: lambda: qk_block(1, 3, wk8, bk_sb, kt_sb),
                13: lambda: qk_block(0, 3, wq8, bq_sb, qt),
            }
            f_s30_b1 = {
                4: lambda: qk_block(1, 2, wq8, bq_sb, qt),
                8: lambda: qk_block(1, 3, wq8, bq_sb, qt),
            }
            f_v = [lambda st=st: v_block(st) for st in range(ST)]

            p = stream(0, 0, None, f_v, f_s00_b1, nxt=(1, 0))
            p = stream(1, 0, p, [], f_s10_b1, nxt=(2, 0))
            p = stream(2, 0, p, [], f_s20_b1, nxt=(3, 0))
            p = stream(3, 0, p, [], f_s30_b1, nxt=(0, 1))
            p = stream(0, 1, p, [], nxt=(1, 1))
            p = stream(1, 1, p, [lambda st=st: out_block(st) for st in range(4)],
                       nxt=(2, 1))
            p = stream(2, 1, p, [lambda st=st: out_block(st) for st in range(4, 8)],
                       nxt=(3, 1))
            p = stream(3, 1, p, [])

            # ---- tail: PV + finish of head (3, qh=1), out-proj of the
            # second q-half interleaved as its halves complete. ------------
            ph, pqh, p_etiles, p_pvp = p
            for it in range(8):            # groups q4 0..3 (bank A)
                pv_8(ph, p_etiles, p_pvp, it)
            finish_group(ph, pqh, p_pvp, 0)
            for it in range(8, ST):        # groups q4 4..7 (bank B)
                pv_8(ph, p_etiles, p_pvp, it)
            for st in range(8, 12):        # q 1024..1535: ready after group A
                out_block_tail(st, "scalar" if st % 2 else "vector")
            finish_group(ph, pqh, p_pvp, 1)
            for st in range(12, 16):
                out_block_tail(st, "scalar" if st % 2 else "vector")

    nc.compile()
    return nc


def _get_graph():
    if "nc" not in _CACHED:
        _CACHED["nc"] = build_core_graph()
    return _CACHED["nc"]


def _pack_dr(a):
    """[1024, M] -> [512, 2*M]: 256-row chunk c packed as [p, j*M + m] with
    [p, j] holding logical row 256c + 128j + p."""
    n, m = a.shape
    return np.ascontiguousarray(
        a.reshape(4, 2, 128, m).transpose(0, 2, 1, 3).reshape(512, 2 * m))


def kernel(x, W_qkv, b_qkv, W_out, b_out):
    import ml_dtypes
    md = ml_dtypes.bfloat16
    f8 = ml_dtypes.float8_e4m3

    x = np.asarray(x, dtype=np.float32)
    W_qkv = np.asarray(W_qkv, dtype=np.float32)
    b_qkv = np.asarray(b_qkv, dtype=np.float32)
    W_out = np.asarray(W_out, dtype=np.float32)
    b_out = np.asarray(b_out, dtype=np.float32)

    nc = _get_graph()

    Wq, Wk, Wv = W_qkv[:, 0:E], W_qkv[:, E:2 * E], W_qkv[:, 2 * E:3 * E]
    bq, bk, bv = b_qkv[0:E], b_qkv[E:2 * E], b_qkv[2 * E:3 * E]

    in_maps = []
    for c in range(8):
        b, hg = c // 4, c % 4
        cols = slice(DQ * hg, DQ * hg + DQ)
        xt_f32 = np.ascontiguousarray(x[b].T)
        in_maps.append({
            "x8a": _pack_dr(xt_f32[:, 0:1024]).astype(f8),
            "x8b": _pack_dr(xt_f32[:, 1024:2048]).astype(f8),
            "wq8": _pack_dr(Wq[:, cols] * WSCALE).astype(f8),
            "wk8": _pack_dr(Wk[:, cols] * WSCALE).astype(f8),
            "wv": np.ascontiguousarray(Wv[:, cols]).astype(md),
            "bq": np.ascontiguousarray(bq[cols].reshape(DQ, 1)) * WSCALE,
            "bk": np.ascontiguousarray(bk[cols].reshape(DQ, 1)) * WSCALE,
            "wo": np.ascontiguousarray(W_out[cols, :]).astype(md),
        })

    res = bass_utils.run_bass_kernel_spmd(nc, in_maps, core_ids=list(range(8)))
    _CACHED["last_results"] = res

    b_eff = (b_out.astype(np.float64) +
             bv.astype(np.float64) @ W_out.astype(np.float64))
    out = np.empty((2, S, E), np.float32)
    for b in range(2):
        acc = np.zeros((S, E), np.float64)
        for hg in range(4):
            acc += res.results[4 * b + hg]["out"].astype(np.float64)
        out[b] = (acc + b_eff).astype(np.float32)
    return out
